# revision 1
# baseline (speedup 1.0000x reference)
"""Multi-Head Latent Attention kernel for 8 Trainium2 NeuronCores.

Sharding: 8 cores = 2 (batch) x 4 (head groups of 4 heads).
Each core computes, for its (batch b, head group g):
  - kv = x_b @ Wc + bc              (replicated small compressor)
  - k,v,q projections for its 4 heads (column-parallel)
  - causal attention for its 4 heads (transpose-free: S^T layout)
  - partial out = y_heads @ Wo[rows of g]   (row-parallel)
Host sums the 4 partials per batch and adds bo.

All matmuls run in bf16 with fp32 PSUM accumulation. Softmax runs
without max-subtraction (scores for this problem are O(1); exp in fp32
is exact enough) so the denominator comes for free from a ones-column
augmented V in the same PSUM accumulation as y.
"""
import sys
import math

sys.path.insert(0, "/opt/trn_rl_repo")

import numpy as np
import ml_dtypes

import concourse.bass as bass
import concourse.tile as tile
from concourse import bacc, mybir
from concourse.bass_utils import run_bass_kernel_spmd

BF16 = ml_dtypes.bfloat16

# Problem shape (hardcoded per contract)
B, T, D = 2, 2048, 1024
H = 16
HD = 64           # head dim
KV = 16           # latent dim
HPC = 4           # heads per core
GD = HPC * HD     # head-group width = 256
NKT = T // 128    # key tiles = 16
SCALE = 1.0 / math.sqrt(HD)

F32 = mybir.dt.float32
BF = mybir.dt.bfloat16

_CACHE = {}


def _build_program():
    nc = bacc.Bacc("TRN2", target_bir_lowering=False, debug=False)

    xT = nc.dram_tensor("xT", [D, T], BF, kind="ExternalInput")
    wq = nc.dram_tensor("wq", [D, GD], BF, kind="ExternalInput")
    bq = nc.dram_tensor("bq", [128, 2], F32, kind="ExternalInput")
    wc = nc.dram_tensor("wc", [128, 8 * KV], BF, kind="ExternalInput")
    bc = nc.dram_tensor("bc", [KV, 1], F32, kind="ExternalInput")
    wk = nc.dram_tensor("wk", [KV, GD], BF, kind="ExternalInput")
    wv = nc.dram_tensor("wv", [KV, GD], BF, kind="ExternalInput")
    bk = nc.dram_tensor("bk", [128, 2], F32, kind="ExternalInput")
    bv = nc.dram_tensor("bv", [1, GD], BF, kind="ExternalInput")
    wo = nc.dram_tensor("wo", [GD, D], BF, kind="ExternalInput")
    tri = nc.dram_tensor("tri", [128, 128], BF, kind="ExternalInput")
    outp = nc.dram_tensor("outp", [T, D], BF, kind="ExternalOutput")

    EXP = mybir.ActivationFunctionType.Exp

    with tile.TileContext(nc) as tc:
        with (
            tc.tile_pool(name="const", bufs=1) as const,
            tc.tile_pool(name="work", bufs=3) as work,
            tc.tile_pool(name="pexps", bufs=16) as pexps,
            tc.tile_pool(name="rbcs", bufs=3) as rbcs,
            tc.tile_pool(name="ostg", bufs=4) as ostg,
            tc.tile_pool(name="rdrams", bufs=2, space="DRAM") as rdrams,
            tc.tile_pool(name="ps", bufs=2, space="PSUM") as ps,
        ):
            # ---- load constants / inputs to SBUF (small consts first so
            # the kv->k/v chain can start as xT tiles stream in) ----
            wc_sb = const.tile([128, 8, KV], BF)
            nc.sync.dma_start(out=wc_sb, in_=wc.ap().rearrange("p (k m) -> p k m", m=KV))
            wk_sb = const.tile([KV, GD], BF)
            nc.sync.dma_start(out=wk_sb, in_=wk.ap())
            wv_sb = const.tile([KV, GD], BF)
            nc.sync.dma_start(out=wv_sb, in_=wv.ap())
            bk_sb = const.tile([128, 2, 1], F32)
            nc.sync.dma_start(out=bk_sb, in_=bk.ap().rearrange("p (c o) -> p c o", o=1))
            bvbc_sb = const.tile([128, GD], BF)
            bv_row = bv.ap()
            bv_bcast = bass.AP(tensor=bv_row.tensor, offset=bv_row.offset,
                               ap=[[0, 128]] + list(bv_row.ap)[1:])
            nc.sync.dma_start(out=bvbc_sb, in_=bv_bcast)
            tri_sb = const.tile([128, 128], BF)
            nc.sync.dma_start(out=tri_sb, in_=tri.ap())
            ones_f = const.tile([1, 64], F32)
            nc.vector.memset(ones_f, 1.0)
            bq_sb = const.tile([128, 2, 1], F32)
            nc.sync.dma_start(out=bq_sb, in_=bq.ap().rearrange("p (c o) -> p c o", o=1))
            bc_sb = const.tile([KV, 1], F32)
            nc.sync.dma_start(out=bc_sb, in_=bc.ap())
            xT_sb = const.tile([128, 8, T], BF)
            xT_r = xT.ap().rearrange("(k p) t -> p k t", p=128)
            wq_sb = const.tile([128, 8, GD], BF)
            wq_r = wq.ap().rearrange("(k p) m -> p k m", p=128)
            dmae = [nc.sync, nc.gpsimd]
            di = 0
            def dput(out, in_):
                nonlocal di
                dmae[di % 2].dma_start(out=out, in_=in_)
                di += 1
            for kt in range(8):
                dput(xT_sb[:, kt, 0:512], xT_r[:, kt, 0:512])
            for kt in range(8):
                dput(xT_sb[:, kt, 512:1024], xT_r[:, kt, 512:1024])
            for kt in range(8):
                dput(wq_sb[:, kt, :], wq_r[:, kt, :])
            for kt in range(8):
                dput(xT_sb[:, kt, 1024:2048], xT_r[:, kt, 1024:2048])
            wo_sb = const.tile([128, 2, D], BF)
            wo_r = wo.ap().rearrange("(k p) n -> p k n", p=128)
            for kt in range(2):
                nc.gpsimd.dma_start(out=wo_sb[:, kt, :], in_=wo_r[:, kt, :])

            kvT_sb = const.tile([KV, T], BF)
            kT_sb = const.tile([128, 2, T], BF)
            qT_sb = const.tile([128, 2, T], BF)
            ynT_sb = const.tile([128, 2, T], BF)
            v_sb = const.tile([128, NKT, HPC, HD + 1], BF)
            nc.vector.memset(v_sb[:, :, :, HD : HD + 1], 1.0)

            def emit_kv_half(g):
                # two n-slices col-packed into one psum tile (strips at
                # partition 0/32) so both accumulations run concurrently;
                # half g covers n = 2g, 2g+1 (only needs xT cols <= 1024(g+1))
                pkv = ps.tile([128, 512], F32, tag="bank", bufs=4, name=f"pkv{g}")
                for kt in range(8):
                    for j in range(2):
                        n = 2 * g + j
                        nc.tensor.matmul(
                            pkv[32 * j : 32 * j + KV, :],
                            lhsT=wc_sb[:, kt, :],
                            rhs=xT_sb[:, kt, n * 512 : n * 512 + 512],
                            start=(kt == 0), stop=(kt == 7),
                            tile_position=(0, 32 * j),
                        )
                for j in range(2):
                    n = 2 * g + j
                    nc.vector.tensor_scalar_add(
                        kvT_sb[0:KV, n * 512 : n * 512 + 512],
                        pkv[32 * j : 32 * j + KV, :], bc_sb)

            def emit_k(c, n):
                ns = slice(n * 512, n * 512 + 512)
                pk = ps.tile([128, 512], F32, tag="bank", bufs=4, name=f"pk{c}{n}")
                nc.tensor.matmul(
                    pk, lhsT=wk_sb[:, c * 128 : (c + 1) * 128], rhs=kvT_sb[:, ns],
                    start=True, stop=True,
                )
                nc.vector.tensor_scalar_add(kT_sb[:, c, ns], pk, bk_sb[:, c, :])

            def emit_q(c, n):
                ns = slice(n * 512, n * 512 + 512)
                pq = ps.tile([128, 512], F32, tag="bank", bufs=4, name=f"pq{c}{n}")
                for kt in range(8):
                    nc.tensor.matmul(
                        pq, lhsT=wq_sb[:, kt, c * 128 : (c + 1) * 128],
                        rhs=xT_sb[:, kt, ns], start=(kt == 0), stop=(kt == 7),
                    )
                nc.vector.tensor_scalar_add(qT_sb[:, c, ns], pq, bq_sb[:, c, :])

            def emit_v(t):
                pv = ps.tile([128, GD], F32, tag="bank", bufs=4, name=f"pv{t}")
                nc.tensor.matmul(
                    pv, lhsT=kvT_sb[:, t * 128 : (t + 1) * 128], rhs=wv_sb,
                    start=True, stop=True,
                )
                nc.vector.tensor_add(
                    out=v_sb[:, t, :, 0:HD],
                    in0=pv.rearrange("p (h d) -> p h d", h=HPC),
                    in1=bvbc_sb.rearrange("p (h d) -> p h d", h=HPC),
                )

            def emit_attn(qc, pair, fillers=None, tail_fillers=None):
                """Causal attention for q window [qc*512, qc*512+512),
                heads 2*pair, 2*pair+1 (PE row-tiled pair). fillers are
                independent emission closures interleaved per ki to keep
                PE busy during score->exp handoffs."""
                fillers = list(fillers or [])
                q0 = qc * 512
                n_ki = 4 * qc + 4
                y_ps = [
                    ps.tile([65, 512], F32, tag="bank", bufs=4,
                            name=f"yps{qc}{pair}{h}")
                    for h in range(2)
                ]
                stride = max(1, -(-n_ki // len(fillers))) if fillers else 0
                for ki in range(n_ki):
                    if fillers and ki % stride == (1 % stride):
                        fillers.pop(0)()
                    vs = max(0, 128 * ki - q0)
                    # both heads' scores side by side in one 2-bank psum
                    s_ps = ps.tile([128, 2, 512], F32, tag="sc", bufs=2,
                                   name=f"s{qc}{pair}{ki}")
                    for h_local in range(2):
                        base = h_local * 64
                        nc.tensor.matmul(
                            s_ps[:, h_local, vs:512],
                            lhsT=kT_sb[base : base + 64, pair,
                                       ki * 128 : (ki + 1) * 128],
                            rhs=qT_sb[base : base + 64, pair, q0 + vs : q0 + 512],
                            start=True, stop=True,
                            tile_position=(base, 0),
                        )
                    px = pexps.tile([128, 2, 512], BF, tag="pexp",
                                    name=f"px{qc}{pair}{ki}")
                    nc.scalar.activation(px[:, :, vs:512], s_ps[:, :, vs:512], EXP)
                    if ki >= 4 * qc:
                        tri_b = bass.AP(tensor=tri_sb.tensor, offset=tri_sb.offset,
                                        ap=[list(tri_sb.ap)[0], [0, 2], [1, 128]])
                        nc.vector.tensor_mul(
                            px[:, :, vs : vs + 128],
                            px[:, :, vs : vs + 128], tri_b,
                        )
                    for h_local in range(2):
                        nc.tensor.matmul(
                            y_ps[h_local][:, vs:512],
                            lhsT=v_sb[:, ki, 2 * pair + h_local, :],
                            rhs=px[:, h_local, vs:512],
                            start=(ki == 0), stop=(ki == n_ki - 1),
                        )
                # drain y psum to SBUF immediately (frees the PSUM slots
                # for the next phase ~5us earlier; makes the normalize mul
                # an all-SBUF fp32 op at 2x DVE rate)
                ysb = work.tile([65, 2, 512], F32, tag="ysb", name=f"ysb{qc}{pair}")
                nc.vector.tensor_copy(out=ysb[:, 0, :], in_=y_ps[0])
                nc.scalar.copy(out=ysb[:, 1, :], in_=y_ps[1])
                sums = work.tile([1, 2, 512], F32, tag="sums", name=f"sums{qc}{pair}")
                nc.vector.tensor_copy(out=sums[0:1, 0, :], in_=ysb[64:65, 0, :])
                nc.scalar.copy(out=sums[0:1, 1, :], in_=ysb[64:65, 1, :])
                recip = work.tile([1, 2, 512], F32, tag="recip", name=f"recip{qc}{pair}")
                nc.vector.reciprocal_approx_fast(out=recip, in_=sums)
                # broadcast recip across 64 partitions per head via a K=1
                # fp32r matmul (full fp32 scalar precision, no DRAM bounce)
                rdram = rdrams.tile([1, 1024], F32, tag="rd", name=f"rd{qc}{pair}")
                nc.sync.dma_start(out=rdram, in_=recip.rearrange("p a b -> p (a b)"))
                rbc = rbcs.tile([64, 2, 512], F32, tag="rbc", name=f"rbc{qc}{pair}")
                bc0 = bass.AP(tensor=rdram.tensor, offset=rdram.offset,
                              ap=[[0, 64], [1, 512]])
                bc1 = bass.AP(tensor=rdram.tensor, offset=rdram.offset + 512,
                              ap=[[0, 64], [1, 512]])
                nc.sync.dma_start(out=rbc[:, 0, :], in_=bc0)
                nc.gpsimd.dma_start(out=rbc[:, 1, :], in_=bc1)
                for f in fillers:
                    f()
                for f in (tail_fillers or []):
                    f()
                for h_local in range(2):
                    nc.vector.tensor_mul(
                        ynT_sb[h_local * 64 : (h_local + 1) * 64, pair,
                               q0 : q0 + 512],
                        ysb[0:64, h_local, :], rbc[:, h_local, :],
                    )

            def emit_outproj(qc, m, n):
                qs = qc * 512 + m * 128
                po = ps.tile([128, 512], F32, tag="bank", bufs=4, name=f"po{qc}{m}{n}")
                for kt in range(2):
                    nc.tensor.matmul(
                        po,
                        lhsT=ynT_sb[:, kt, qs : qs + 128],
                        rhs=wo_sb[:, kt, n * 512 : (n + 1) * 512],
                        start=(kt == 0), stop=(kt == 1),
                    )
                st = ostg.tile([128, 512], BF, tag="ostg", name=f"ost{qc}{m}{n}")
                if qc == 3 and (m + n) % 2 == 0:
                    nc.scalar.copy(st, po)
                else:
                    nc.vector.tensor_copy(st, po)
                if qc == 3:
                    eng = [nc.sync, nc.gpsimd, nc.scalar][(2 * m + n) % 3]
                else:
                    eng = nc.sync if (m + n) % 2 == 0 else nc.gpsimd
                eng.dma_start(
                    out=outp.ap()[qs : qs + 128, n * 512 : (n + 1) * 512],
                    in_=st,
                )

            # ---- emission schedule ----
            # proj_group(n): projections for the n-th 512-token slab.
            # group(n) is needed by attention window qc=n; emit group(0)
            # up front, group(n) as fillers inside window n-1. out-proj
            # of window qc streams as fillers inside window qc+1.
            def proj_group(n):
                units = []
                if n == 0:
                    units.append(lambda: emit_kv_half(0))
                elif n == 2:
                    units.append(lambda: emit_kv_half(1))
                units += [lambda c=c, n=n: emit_k(c, n) for c in range(2)]
                units += [lambda c=c, n=n: emit_q(c, n) for c in range(2)]
                units += [lambda t=t: emit_v(t) for t in range(4 * n, 4 * n + 4)]
                return units

            def outproj_group(qc):
                return [
                    lambda m=m, n=n: emit_outproj(qc, m, n)
                    for m in range(4) for n in range(2)
                ]

            for u in proj_group(0):
                u()
            g1 = proj_group(1)
            emit_attn(0, 0, fillers=g1[:5])
            emit_attn(0, 1, fillers=g1[5:])
            emit_attn(1, 0, fillers=proj_group(2))
            emit_attn(1, 1, fillers=proj_group(3))
            og0 = outproj_group(0)
            og1 = outproj_group(1)
            emit_attn(2, 0, fillers=og0[:6], tail_fillers=og0[6:])
            emit_attn(2, 1, fillers=og1[:6], tail_fillers=og1[6:])
            og2 = outproj_group(2)
            emit_attn(3, 0, fillers=og2[:4])
            emit_attn(3, 1, tail_fillers=og2[4:])
            for u in outproj_group(3):
                u()

    nc.compile()
    return nc


def _prep_inputs(inputs):
    """Host-side shard prep: per-core input dicts."""
    x = np.asarray(inputs["x"], np.float32)
    Wc = np.asarray(inputs["Wc"], np.float32)
    bc = np.asarray(inputs["bc"], np.float32)
    Wk = np.asarray(inputs["Wk"], np.float32)
    bk = np.asarray(inputs["bk"], np.float32)
    Wv = np.asarray(inputs["Wv"], np.float32)
    bv = np.asarray(inputs["bv"], np.float32)
    Wq = np.asarray(inputs["Wq"], np.float32)
    bq = np.asarray(inputs["bq"], np.float32)
    Wo = np.asarray(inputs["Wo"], np.float32)

    tri = np.triu(np.ones((128, 128), np.float32)).astype(BF16)  # key r <= q c
    wc_b = np.ascontiguousarray(
        Wc.reshape(8, 128, KV).transpose(1, 0, 2).reshape(128, 8 * KV)).astype(BF16)
    bc_b = bc.reshape(KV, 1).astype(np.float32)

    xT = [np.ascontiguousarray(x[b].T).astype(BF16) for b in range(B)]

    in_maps = []
    for core in range(8):
        b, g = core // 4, core % 4
        gsl = slice(g * GD, (g + 1) * GD)
        in_maps.append({
            "xT": xT[b],
            "wq": np.ascontiguousarray(Wq[:, gsl] * SCALE).astype(BF16),
            "bq": np.ascontiguousarray((bq[gsl] * SCALE).reshape(2, 128).T).astype(np.float32),
            "wc": wc_b,
            "bc": bc_b,
            "wk": np.ascontiguousarray(Wk[:, gsl]).astype(BF16),
            "wv": np.ascontiguousarray(Wv[:, gsl]).astype(BF16),
            "bk": np.ascontiguousarray(bk[gsl].reshape(2, 128).T).astype(np.float32),
            "bv": np.ascontiguousarray(bv[gsl]).reshape(1, GD).astype(BF16),
            "wo": np.ascontiguousarray(Wo[gsl, :]).astype(BF16),
            "tri": tri,
        })
    return in_maps


def run(inputs, trace=False, tmpdir=None):
    if "nc" not in _CACHE:
        _CACHE["nc"] = _build_program()
    nc = _CACHE["nc"]
    in_maps = _prep_inputs(inputs)

    kwargs = {}
    if trace:
        # NTFF profiling under axon needs the antenv.axon_hooks bridge;
        # shim it if the image lacks it.
        try:
            import antenv.axon_hooks  # noqa: F401
        except ImportError:
            import types
            import antenv  # noqa: F401
            from trn_agent_boot.trn_boot import _ntff_profile_via_ctypes
            hook = _ntff_profile_via_ctypes("/opt/axon/libaxon_pjrt.so")
            mod = types.ModuleType("antenv.axon_hooks")
            mod.get_axon_ntff_profile_hook = lambda: hook
            sys.modules["antenv.axon_hooks"] = mod
        kwargs = dict(trace=True, tmpdir=tmpdir)

    res = run_bass_kernel_spmd(nc, in_maps, list(range(8)), **kwargs)

    bo = np.asarray(inputs["bo"], np.float32)
    out = np.zeros((B, T, D), np.float32)
    for core in range(8):
        out[core // 4] += res.results[core]["outp"].astype(np.float32)
    out += bo
    return out, res


def kernel(**inputs):
    out, _ = run(inputs, trace=False)
    return out

